# revision 7
# baseline (speedup 1.0000x reference)
"""Trainium2 Bass kernel for nn_BertEncoder_403726926494.

Reference computation (per batch element):
  - ragged sentence extraction from hidden_states, masked-softmax attention
    pooling per sentence with W_doc            -> doc_pooled [B, D, H]
  - query extraction (rows 1..32), masked-softmax pooling with W_query
    broadcast over D                           -> q_bcast   [B, D, H]

Device strategy (SPMD, one program on 8 cores, 8 batch elements per core):
  - The host packs, per core, a dense token stream: 8x32 query rows
    (chunks 0..1) followed by each example's contiguous sentence region
    rows [ql+2, ql+2+sum(seq_lens)), concatenated back-to-back, zero-pad
    to T*128 rows.  Stored bf16 as [128, T, 770]: col 768 = 1.0 (ones
    column for the softmax denominators), col 769 = pad.
  - A host-built 0/1 selector sel[token, m] (bf16, [128, T, 128]) maps
    every token to its output row m: cols 0..7 = the 8 queries (masked
    to ql), cols 8.. = every non-empty sentence of the core's examples.
  - Per chunk c (128 tokens):
      scores  s = x . W  (W_query for chunks 0..1, W_doc for the rest)
              one DVE tensor_tensor_reduce (mult + add-accum)
      es      = exp(s)            (ACT, [128,1] per chunk)
      at      = sel * es          (DVE tensor_scalar, per-partition scalar)
      acc    += at^T @ x[:, 0:769]  (PE matmul, K=128, M=128, N=769,
              accumulated over all T chunks in one PSUM region; the ones
              column makes acc[:, 768] the softmax denominator)
  - out[m] = acc[m, 0:768] / (acc[m, 768] + eps); eps keeps unused rows
    at 0.  One [128, 768] bf16 store; host scatters rows to (e, j)/query.
  - b_doc / b_query shift every score in a softmax segment equally, so
    they cancel and are ignored.  exp() without max-subtraction is safe:
    |s| <~ 3 for this data distribution.
"""

import numpy as np
import ml_dtypes

B, L, H = 64, 512, 768
D, S, Q = 16, 64, 32
NCORES = 8
EX_PER_CORE = 8
QCH = 2  # query chunks: 8 examples x 32 rows = 256 = 2*128
HP = H + 2  # 770: ones col at 768, pad col at 769 (4-byte-aligned chunks)
DEN_EPS = 1.0e-30
BF16 = ml_dtypes.bfloat16
SPLIT_MM = True  # True: split each chunk matmul at col 512 (PSUM banks)

_compiled: dict = {}


def _build(T):
    from contextlib import ExitStack

    import concourse.bacc as bacc
    import concourse.tile as tile
    from concourse import mybir

    f32 = mybir.dt.float32
    bf16 = mybir.dt.bfloat16
    MULT = mybir.AluOpType.mult
    ADD = mybir.AluOpType.add
    EXP = mybir.ActivationFunctionType.Exp
    COPY = mybir.ActivationFunctionType.Copy

    nc = bacc.Bacc(
        "TRN2", target_bir_lowering=False, debug=False, num_devices=NCORES
    )

    xs_d = nc.dram_tensor("xs", [128, T, HP], bf16, kind="ExternalInput").ap()
    sel_d = nc.dram_tensor("sel", [128, T, 128], bf16, kind="ExternalInput").ap()
    wb_d = nc.dram_tensor("wb", [2, H], bf16, kind="ExternalInput").ap()
    out_d = nc.dram_tensor("out", [128, H], bf16, kind="ExternalOutput").ap()

    with tile.TileContext(nc) as tc, ExitStack() as ctx:
        const = ctx.enter_context(tc.tile_pool(name="const", bufs=1))
        wb = const.tile([128, 2, H], bf16)
        for b in range(2):
            nc.scalar.dma_start(
                out=wb[:, b, :], in_=wb_d[b : b + 1, :].broadcast_to([128, H])
            )
        sel = const.tile([128, T, 128], bf16)
        nc.scalar.dma_start(out=sel[:], in_=sel_d[:])

        xp = ctx.enter_context(tc.tile_pool(name="xp", bufs=1))
        xs = xp.tile([128, T, HP], bf16)
        for c0 in range(0, T, 2):
            c1 = min(c0 + 2, T)
            nc.sync.dma_start(out=xs[:, c0:c1, :], in_=xs_d[:, c0:c1, :])

        sc = ctx.enter_context(tc.tile_pool(name="sc", bufs=1))
        scol = sc.tile([128, T], f32)
        es = sc.tile([128, T], f32)
        at = sc.tile([128, T, 128], bf16)
        xw = sc.tile([128, T, H], bf16)
        psum = ctx.enter_context(tc.tile_pool(name="ps", bufs=1, space="PSUM"))
        acc = psum.tile([128, 1024], f32)

        def emit_mm(c):
            first, last = c == 0, c == T - 1
            nc.tensor.matmul(
                acc[:, 0:512], at[:, c, :], xs[:, c, 0:512],
                start=first, stop=last,
            )
            nc.tensor.matmul(
                acc[:, 512 : H + 1], at[:, c, :], xs[:, c, 512 : H + 1],
                start=first, stop=last,
            )

        for c0 in range(0, T, 2):
            c1 = min(c0 + 2, T)
            for c in range(c0, c1):
                w = wb[:, 1 if c < QCH else 0, :]
                nc.vector.tensor_tensor(
                    out=xw[:, c, :], in0=xs[:, c, 0:H], in1=w, op=MULT
                )
            nc.vector.tensor_reduce(
                out=scol[:, c0:c1],
                in_=xw[:, c0:c1, :],
                axis=mybir.AxisListType.X,
                op=ADD,
            )
            nc.scalar.activation(es[:, c0:c1], scol[:, c0:c1], EXP)
            for c in range(c0, c1):
                nc.gpsimd.tensor_scalar(
                    out=at[:, c, :],
                    in0=sel[:, c, :],
                    scalar1=es[:, c : c + 1],
                    scalar2=None,
                    op0=MULT,
                )
                emit_mm(c)

        de = sc.tile([128, 1], f32)
        nc.vector.tensor_scalar(
            out=de[:], in0=acc[:, H : H + 1], scalar1=DEN_EPS,
            scalar2=None, op0=ADD,
        )
        rec = sc.tile([128, 1], f32)
        nc.vector.reciprocal(rec[:], de[:])
        do = sc.tile([128, H], bf16)
        nc.scalar.activation(do[:], acc[:, 0:H], COPY, bias=0.0, scale=rec[:, 0:1])
        nc.scalar.dma_start(out=out_d[:], in_=do[:])

    nc.compile()
    return nc


def _prepare(query_len, seq_lens):
    """Assign examples to cores (balanced stream length) and compute T."""
    ql = np.asarray(query_len).astype(np.int64)
    sl = np.asarray(seq_lens).astype(np.int64)
    S = sl.sum(axis=1)
    dl = (sl > 0).sum(axis=1)
    order = np.argsort(-S, kind="stable")
    loads = np.zeros(NCORES, np.int64)
    counts = np.zeros(NCORES, np.int64)
    assign = [[] for _ in range(NCORES)]
    for e in order:
        cand = [c for c in range(NCORES) if counts[c] < EX_PER_CORE]
        c = min(cand, key=lambda k: loads[k])
        assign[c].append(int(e))
        loads[c] += int(S[e])
        counts[c] += 1
    T = QCH + int((int(loads.max()) + 127) // 128)
    for c in range(NCORES):
        m_used = EX_PER_CORE + int(dl[assign[c]].sum())
        assert m_used <= 128, f"core {c} needs {m_used} output rows"
    return assign, T, ql, sl, S, dl


def _pack_core(hs, examples, T, ql, sl, S, dl):
    """Build one core's packed stream, selector and output-row maps."""
    rows = T * 128
    xsh = np.zeros((rows, HP), np.float32)
    xsh[:, H] = 1.0
    sel = np.zeros((rows, 128), np.float32)
    sent_rows = {}  # (e, j) -> output row m
    q_rows = {}  # e -> output row m
    mcol = EX_PER_CORE
    pos = QCH * 128
    for i, e in enumerate(examples):
        r0 = 32 * i
        xsh[r0 : r0 + 32, 0:H] = hs[e, 1 : 1 + Q, :]
        sel[r0 : r0 + int(ql[e]), i] = 1.0
        q_rows[e] = i
        ns = int(S[e])
        base = int(ql[e]) + 2
        xsh[pos : pos + ns, 0:H] = hs[e, base : base + ns, :]
        off = 0
        for j in range(int(dl[e])):
            ln = int(sl[e, j])
            sel[pos + off : pos + off + ln, mcol] = 1.0
            sent_rows[(e, j)] = mcol
            off += ln
            mcol += 1
        pos += ns
    xs = np.ascontiguousarray(
        xsh.reshape(T, 128, HP).transpose(1, 0, 2)
    ).astype(BF16)
    selr = np.ascontiguousarray(
        sel.reshape(T, 128, 128).transpose(1, 0, 2)
    ).astype(BF16)
    return xs, selr, q_rows, sent_rows


def kernel(hidden_states, W_doc, b_doc, W_query, b_query, query_len, seq_lens):
    hs = np.ascontiguousarray(np.asarray(hidden_states, dtype=np.float32))
    wd = np.asarray(W_doc, np.float32).reshape(H)
    wq = np.asarray(W_query, np.float32).reshape(H)

    assign, T, ql, sl, S, dl = _prepare(query_len, seq_lens)

    nc = _compiled.get(T)
    if nc is None:
        nc = _build(T)
        _compiled[T] = nc

    wb = np.stack([wd, wq]).astype(BF16)  # [2, H]; device broadcasts

    in_maps = []
    maps = []
    for c in range(NCORES):
        xs, selr, q_rows, sent_rows = _pack_core(
            hs, assign[c], T, ql, sl, S, dl
        )
        in_maps.append({"xs": xs, "sel": selr, "wb": wb})
        maps.append((q_rows, sent_rows))

    from concourse.bass_utils import run_bass_kernel_spmd

    res = run_bass_kernel_spmd(nc, in_maps, list(range(NCORES)))

    doc = np.zeros((B, D, H), np.float32)
    qp = np.empty((B, H), np.float32)
    for c in range(NCORES):
        r = np.asarray(res.results[c]["out"], dtype=np.float32)
        q_rows, sent_rows = maps[c]
        for e, m in q_rows.items():
            qp[e] = r[m]
        for (e, j), m in sent_rows.items():
            doc[e, j] = r[m]
    q_bcast = np.broadcast_to(qp[:, None, :], (B, D, H))
    return doc, q_bcast


# revision 9
# speedup vs baseline: 1.3871x; 1.3871x over previous
"""Trainium2 Bass kernel for nn_BertEncoder_403726926494.

Reference computation (per batch element):
  - ragged sentence extraction from hidden_states, masked-softmax attention
    pooling per sentence with W_doc            -> doc_pooled [B, D, H]
  - query extraction (rows 1..32), masked-softmax pooling with W_query
    broadcast over D                           -> q_bcast   [B, D, H]

Device strategy (SPMD, one program on 8 cores, 8 batch elements per core):
  - The host packs, per core, a dense bf16 token stream [128, T, 770]:
    first the 8 examples' query rows (rows 1..1+ql each, tightly packed
    into a QB-row block, QB = max over cores, zero-padded), then each
    example's contiguous sentence region rows [ql+2, ql+2+sum(seq_lens)),
    back-to-back.  Col 768 = 1.0 (ones column -> softmax denominators),
    col 769 = pad for 4-byte alignment.
  - A host-built 0/1 selector sel[token, m] ([128, T, M], M=96) maps every
    token to its output row m: cols 0..7 = the 8 queries, cols 8.. = every
    non-empty sentence.  Ragged masking is entirely in the selector.
  - Per chunk c (128 tokens):
      scores  s = x . W  (W_query in the query block, W_doc after; the
              one boundary chunk gets two partition-range multiplies)
              TT mult on DVE (paired chunks where possible), then a
              sum-reduce: DVE tensor_scalar(accum_out) for 2 of 3 chunks,
              ACT Copy(accum_out) for the rest (engine balance)
      es      = exp(s)          (ACT, paired chunks)
      at      = sel * es        (ACT Copy with per-partition scale)
      acc    += at^T @ x        (PE, K=128, M=96, N=769, split 512+257,
              accumulated over all T chunks in one PSUM region)
  - out[m] = acc[m, 0:768] / (acc[m, 768] + eps); eps keeps unused rows
    at 0.  One [96, 768] bf16 store; host scatters rows to (e, j)/query.
  - b_doc / b_query shift every score in a softmax segment equally, so
    they cancel and are ignored.  exp() without max-subtraction is safe:
    |s| <~ 3 for this data distribution.
"""

import numpy as np
import ml_dtypes

B, L, H = 64, 512, 768
D, S, Q = 16, 64, 32
NCORES = 8
EX_PER_CORE = 8
HP = H + 2  # 770: ones col at 768, pad col at 769 (4-byte-aligned chunks)
M_OUT = 96  # selector / output rows (8 queries + up to 88 sentences)
DEN_EPS = 1.0e-30
BF16 = ml_dtypes.bfloat16
FP8 = ml_dtypes.float8_e4m3fn
SEL_FP8 = True  # selector dtype (fp8 halves its DMA); flip off if probes fail
AT_ON_ACT = True  # at = sel*es on ACT (Copy w/ scale) vs DVE tensor_scalar
DVE_RED_PATTERN = (True, True, False)  # per chunk c: DVE ts-reduce if [c%3]

_compiled: dict = {}


def _build(T, QB):
    from contextlib import ExitStack

    import concourse.bacc as bacc
    import concourse.tile as tile
    from concourse import mybir

    f32 = mybir.dt.float32
    bf16 = mybir.dt.bfloat16
    f8 = mybir.dt.float8e4
    MULT = mybir.AluOpType.mult
    ADD = mybir.AluOpType.add
    EXP = mybir.ActivationFunctionType.Exp
    COPY = mybir.ActivationFunctionType.Copy

    seldt_d, seldt_h = (f8, FP8) if SEL_FP8 else (bf16, BF16)

    nc = bacc.Bacc(
        "TRN2", target_bir_lowering=False, debug=False, num_devices=NCORES
    )

    xs_d = nc.dram_tensor("xs", [128, T, HP], bf16, kind="ExternalInput").ap()
    sel_d = nc.dram_tensor(
        "sel", [128, T, M_OUT], seldt_d, kind="ExternalInput"
    ).ap()
    wb_d = nc.dram_tensor("wb", [2, H], bf16, kind="ExternalInput").ap()
    out_d = nc.dram_tensor("out", [M_OUT, H], bf16, kind="ExternalOutput").ap()

    qch, qr = divmod(QB, 128)  # full query chunks, boundary partition

    with tile.TileContext(nc) as tc, ExitStack() as ctx:
        const = ctx.enter_context(tc.tile_pool(name="const", bufs=1))
        wb = const.tile([128, 2, H], bf16)
        for b in range(2):
            nc.scalar.dma_start(
                out=wb[:, b, :], in_=wb_d[b : b + 1, :].broadcast_to([128, H])
            )
        sel = const.tile([128, T, M_OUT], seldt_d)
        nc.scalar.dma_start(out=sel[:], in_=sel_d[:])

        xp = ctx.enter_context(tc.tile_pool(name="xp", bufs=1))
        xs = xp.tile([128, T, HP], bf16)
        for c0 in range(0, T, 2):
            c1 = min(c0 + 2, T)
            nc.sync.dma_start(out=xs[:, c0:c1, :], in_=xs_d[:, c0:c1, :])

        sc = ctx.enter_context(tc.tile_pool(name="sc", bufs=1))
        scol = sc.tile([128, T], f32)
        es = sc.tile([128, T], f32)
        at = sc.tile([128, T, M_OUT], bf16)
        xw = sc.tile([128, T, H], bf16)
        xdump = sc.tile([128, H], bf16)  # mandatory out of ts-reduce
        psum = ctx.enter_context(tc.tile_pool(name="ps", bufs=1, space="PSUM"))
        acc = psum.tile([128, 1024], f32)

        def wsel(c):
            return 1 if c < qch or (c == qch and qr > 0) else 0

        def emit_tt(c0, c1):
            """TT mults for chunks [c0, c1); pairs same-w chunks."""
            c = c0
            while c < c1:
                if c == qch and qr > 0:
                    # boundary chunk: query rows [0:qr], doc rows [qr:128]
                    nc.vector.tensor_tensor(
                        out=xw[0:qr, c, :], in0=xs[0:qr, c, 0:H],
                        in1=wb[0:qr, 1, :], op=MULT,
                    )
                    nc.vector.tensor_tensor(
                        out=xw[qr:128, c, :], in0=xs[qr:128, c, 0:H],
                        in1=wb[qr:128, 0, :], op=MULT,
                    )
                    c += 1
                    continue
                w = wsel(c)
                if c + 1 < c1 and (c + 1 != qch or qr == 0) and wsel(c + 1) == w:
                    nc.vector.tensor_tensor(
                        out=xw[:, c : c + 2, :],
                        in0=xs[:, c : c + 2, 0:H],
                        in1=wb[:, w, :]
                        .rearrange("p (o h) -> p o h", o=1)
                        .broadcast_to([128, 2, H]),
                        op=MULT,
                    )
                    c += 2
                else:
                    nc.vector.tensor_tensor(
                        out=xw[:, c, :], in0=xs[:, c, 0:H],
                        in1=wb[:, w, :], op=MULT,
                    )
                    c += 1

        def emit_reduce(c):
            if DVE_RED_PATTERN[c % len(DVE_RED_PATTERN)]:
                nc.vector.tensor_scalar(
                    out=xdump[:], in0=xw[:, c, :], scalar1=1.0, scalar2=0.0,
                    op0=MULT, op1=ADD, accum_out=scol[:, c : c + 1],
                )
            else:
                nc.scalar.activation(
                    xdump[:], xw[:, c, :], COPY,
                    accum_out=scol[:, c : c + 1],
                )

        def emit_at(c):
            if AT_ON_ACT:
                nc.scalar.activation(
                    at[:, c, :], sel[:, c, :], COPY,
                    bias=0.0, scale=es[:, c : c + 1],
                )
            else:
                nc.vector.tensor_scalar(
                    out=at[:, c, :], in0=sel[:, c, :],
                    scalar1=es[:, c : c + 1], scalar2=None, op0=MULT,
                )

        def emit_mm(c):
            first, last = c == 0, c == T - 1
            nc.tensor.matmul(
                acc[0:M_OUT, 0:512], at[:, c, :], xs[:, c, 0:512],
                start=first, stop=last,
            )
            nc.tensor.matmul(
                acc[0:M_OUT, 512 : H + 1], at[:, c, :], xs[:, c, 512 : H + 1],
                start=first, stop=last,
            )

        for c0 in range(0, T, 2):
            c1 = min(c0 + 2, T)
            emit_tt(c0, c1)
            for c in range(c0, c1):
                emit_reduce(c)
            nc.scalar.activation(es[:, c0:c1], scol[:, c0:c1], EXP)
            for c in range(c0, c1):
                emit_at(c)
                emit_mm(c)

        de = sc.tile([M_OUT, 1], f32)
        nc.vector.tensor_scalar(
            out=de[:], in0=acc[0:M_OUT, H : H + 1], scalar1=DEN_EPS,
            scalar2=None, op0=ADD,
        )
        rec = sc.tile([M_OUT, 1], f32)
        nc.vector.reciprocal(rec[:], de[:])
        do = sc.tile([M_OUT, H], bf16)
        nc.scalar.activation(
            do[:], acc[0:M_OUT, 0:H], COPY, bias=0.0, scale=rec[:, 0:1]
        )
        nc.scalar.dma_start(out=out_d[:], in_=do[:])

    nc.compile()
    return nc


def _prepare(query_len, seq_lens):
    """Assign examples to cores (balanced stream length); compute T, QB."""
    ql = np.asarray(query_len).astype(np.int64)
    sl = np.asarray(seq_lens).astype(np.int64)
    S = sl.sum(axis=1)
    dl = (sl > 0).sum(axis=1)

    def greedy(metric):
        order = np.argsort(-metric, kind="stable")
        loads = np.zeros(NCORES, np.int64)
        counts = np.zeros(NCORES, np.int64)
        assign = [[] for _ in range(NCORES)]
        for e in order:
            cand = [c for c in range(NCORES) if counts[c] < EX_PER_CORE]
            c = min(cand, key=lambda k: loads[k])
            assign[c].append(int(e))
            loads[c] += int(metric[e])
            counts[c] += 1
        # query block size: 32-aligned (DVE partition offsets must be)
        QB = -(-max(int(ql[a].sum()) for a in assign) // 32) * 32
        T = max(int((QB + int(S[a].sum()) + 127) // 128) for a in assign)
        return assign, T, QB

    best = None
    for metric in (S + ql, S):
        assign, T, QB = greedy(metric)
        if best is None or (T, QB) < (best[1], best[2]):
            best = (assign, T, QB)
    assign, T, QB = best
    for c in range(NCORES):
        m_used = EX_PER_CORE + int(dl[assign[c]].sum())
        assert m_used <= M_OUT, f"core {c} needs {m_used} output rows"
    return assign, T, QB, ql, sl, S, dl


def _pack_core(hs, examples, T, QB, ql, sl, S, dl):
    """Build one core's packed stream, selector and output-row maps."""
    rows = T * 128
    xsh = np.zeros((rows, HP), np.float32)
    xsh[:, H] = 1.0
    sel = np.zeros((rows, M_OUT), np.float32)
    sent_rows = {}  # (e, j) -> output row m
    q_rows = {}  # e -> output row m
    mcol = EX_PER_CORE
    qpos = 0
    pos = QB
    for i, e in enumerate(examples):
        nq = int(ql[e])
        xsh[qpos : qpos + nq, 0:H] = hs[e, 1 : 1 + nq, :]
        sel[qpos : qpos + nq, i] = 1.0
        q_rows[e] = i
        qpos += nq
        ns = int(S[e])
        base = int(ql[e]) + 2
        xsh[pos : pos + ns, 0:H] = hs[e, base : base + ns, :]
        off = 0
        for j in range(int(dl[e])):
            ln = int(sl[e, j])
            sel[pos + off : pos + off + ln, mcol] = 1.0
            sent_rows[(e, j)] = mcol
            off += ln
            mcol += 1
        pos += ns
    seldt = FP8 if SEL_FP8 else BF16
    xs = np.ascontiguousarray(
        xsh.reshape(T, 128, HP).transpose(1, 0, 2)
    ).astype(BF16)
    selr = np.ascontiguousarray(
        sel.reshape(T, 128, M_OUT).transpose(1, 0, 2)
    ).astype(seldt)
    return xs, selr, q_rows, sent_rows


def kernel(hidden_states, W_doc, b_doc, W_query, b_query, query_len, seq_lens):
    hs = np.ascontiguousarray(np.asarray(hidden_states, dtype=np.float32))
    wd = np.asarray(W_doc, np.float32).reshape(H)
    wq = np.asarray(W_query, np.float32).reshape(H)

    assign, T, QB, ql, sl, S, dl = _prepare(query_len, seq_lens)

    nc = _compiled.get((T, QB))
    if nc is None:
        nc = _build(T, QB)
        _compiled[(T, QB)] = nc

    wb = np.stack([wd, wq]).astype(BF16)  # [2, H]; device broadcasts

    in_maps = []
    maps = []
    for c in range(NCORES):
        xs, selr, q_rows, sent_rows = _pack_core(
            hs, assign[c], T, QB, ql, sl, S, dl
        )
        in_maps.append({"xs": xs, "sel": selr, "wb": wb})
        maps.append((q_rows, sent_rows))

    from concourse.bass_utils import run_bass_kernel_spmd

    res = run_bass_kernel_spmd(nc, in_maps, list(range(NCORES)))

    doc = np.zeros((B, D, H), np.float32)
    qp = np.empty((B, H), np.float32)
    for c in range(NCORES):
        r = np.asarray(res.results[c]["out"], dtype=np.float32)
        q_rows, sent_rows = maps[c]
        for e, m in q_rows.items():
            qp[e] = r[m]
        for (e, j), m in sent_rows.items():
            doc[e, j] = r[m]
    q_bcast = np.broadcast_to(qp[:, None, :], (B, D, H))
    return doc, q_bcast


# revision 12
# speedup vs baseline: 1.5595x; 1.1243x over previous
"""Trainium2 Bass kernel for nn_BertEncoder_403726926494.

Reference computation (per batch element):
  - ragged sentence extraction from hidden_states, masked-softmax attention
    pooling per sentence with W_doc            -> doc_pooled [B, D, H]
  - query extraction (rows 1..32), masked-softmax pooling with W_query
    broadcast over D                           -> q_bcast   [B, D, H]

Device strategy (SPMD, one program on 8 cores, 8 batch elements per core):
  - The host packs, per core, a dense bf16 token stream [128, T, 770]:
    first the 8 examples' query rows (rows 1..1+ql each, tightly packed
    into a QB-row block, QB = max over cores, zero-padded), then each
    example's contiguous sentence region rows [ql+2, ql+2+sum(seq_lens)),
    back-to-back.  Col 768 = 1.0 (ones column -> softmax denominators),
    col 769 = pad for 4-byte alignment.
  - A host-built 0/1 selector sel[token, m] ([128, T, M], M=96) maps every
    token to its output row m: cols 0..7 = the 8 queries, cols 8.. = every
    non-empty sentence.  Ragged masking is entirely in the selector.
  - Per chunk c (128 tokens):
      scores  s = x . W  (W_query in the query block, W_doc after; the
              one boundary chunk gets two partition-range multiplies)
              TT mult on DVE (paired chunks where possible), then a
              sum-reduce: DVE tensor_scalar(accum_out) for 2 of 3 chunks,
              ACT Copy(accum_out) for the rest (engine balance)
      es      = exp(s)          (ACT, paired chunks)
      at      = sel * es        (ACT Copy with per-partition scale)
      acc    += at^T @ x        (PE, K=128, M=96, N=769, split 512+257,
              accumulated over all T chunks in one PSUM region)
  - out[m] = acc[m, 0:768] / (acc[m, 768] + eps); eps keeps unused rows
    at 0.  One [96, 768] bf16 store; host scatters rows to (e, j)/query.
  - b_doc / b_query shift every score in a softmax segment equally, so
    they cancel and are ignored.  exp() without max-subtraction is safe:
    |s| <~ 3 for this data distribution.
"""

import numpy as np
import ml_dtypes

B, L, H = 64, 512, 768
D, S, Q = 16, 64, 32
NCORES = 8
EX_PER_CORE = 8
HP = H + 2  # 770: ones col at 768, pad col at 769 (4-byte-aligned chunks)
M_OUT = 96  # selector / output rows (8 queries + up to 88 sentences)
DEN_EPS = 1.0e-30
BF16 = ml_dtypes.bfloat16
FP8 = ml_dtypes.float8_e4m3fn
SEL_FP8 = False  # selector dtype (fp8 halves its DMA)
AT_ON_ACT = True  # at = sel*es on ACT (Copy w/ scale) vs DVE tensor_scalar
# Score pipeline per chunk: fused STT on DVE (1x mode but one pass) vs
# TT mult on DVE (2x) + Copy-accum reduce on ACT.  Pattern by c % len.
STT_PATTERN = (True, True, False)  # True -> fused STT on DVE

_compiled: dict = {}


def _build(T, QB):
    from contextlib import ExitStack

    import concourse.bacc as bacc
    import concourse.tile as tile
    from concourse import mybir

    f32 = mybir.dt.float32
    bf16 = mybir.dt.bfloat16
    f8 = mybir.dt.float8e4
    MULT = mybir.AluOpType.mult
    ADD = mybir.AluOpType.add
    EXP = mybir.ActivationFunctionType.Exp
    COPY = mybir.ActivationFunctionType.Copy

    seldt_d, seldt_h = (f8, FP8) if SEL_FP8 else (bf16, BF16)

    nc = bacc.Bacc(
        "TRN2", target_bir_lowering=False, debug=False, num_devices=NCORES
    )

    xs_d = nc.dram_tensor("xs", [128, T, HP], bf16, kind="ExternalInput").ap()
    sel_d = nc.dram_tensor(
        "sel", [128, T, M_OUT], seldt_d, kind="ExternalInput"
    ).ap()
    wb_d = nc.dram_tensor("wb", [2, H], bf16, kind="ExternalInput").ap()
    out_d = nc.dram_tensor("out", [M_OUT, H], bf16, kind="ExternalOutput").ap()

    qch, qr = divmod(QB, 128)  # full query chunks, boundary partition

    with tile.TileContext(nc) as tc, ExitStack() as ctx:
        const = ctx.enter_context(tc.tile_pool(name="const", bufs=1))
        wb = const.tile([128, 2, H], bf16)
        for b in range(2):
            nc.scalar.dma_start(
                out=wb[:, b, :], in_=wb_d[b : b + 1, :].broadcast_to([128, H])
            )
        sel = const.tile([128, T, M_OUT], seldt_d)
        nc.scalar.dma_start(out=sel[:], in_=sel_d[:])

        xp = ctx.enter_context(tc.tile_pool(name="xp", bufs=1))
        xs = xp.tile([128, T, HP], bf16)
        for c0 in range(0, T, 2):
            c1 = min(c0 + 2, T)
            nc.sync.dma_start(out=xs[:, c0:c1, :], in_=xs_d[:, c0:c1, :])

        sc = ctx.enter_context(tc.tile_pool(name="sc", bufs=1))
        scol = sc.tile([128, T], f32)
        es = sc.tile([128, T], f32)
        at = sc.tile([128, T, M_OUT], bf16)
        xw = sc.tile([128, T, H], bf16)
        xdump = sc.tile([128, H], bf16)  # mandatory out of ts-reduce
        psum = ctx.enter_context(tc.tile_pool(name="ps", bufs=1, space="PSUM"))
        acc = psum.tile([128, 1024], f32)

        def wsel(c):
            return 1 if c < qch or (c == qch and qr > 0) else 0

        def emit_scores(c):
            """Scores for chunk c: fused STT (DVE) or TT (DVE) + red (ACT).

            The query/doc boundary chunk always takes the TT+ACT path:
            a partition-split STT would cost two full 1x passes."""
            if c == qch and qr > 0:
                nc.vector.tensor_tensor(
                    out=xw[0:qr, c, :], in0=xs[0:qr, c, 0:H],
                    in1=wb[0:qr, 1, :], op=MULT,
                )
                nc.vector.tensor_tensor(
                    out=xw[qr:128, c, :], in0=xs[qr:128, c, 0:H],
                    in1=wb[qr:128, 0, :], op=MULT,
                )
                nc.scalar.activation(
                    xdump[:], xw[:, c, :], COPY,
                    accum_out=scol[:, c : c + 1],
                )
                return
            w = wb[:, wsel(c), :]
            if STT_PATTERN[c % len(STT_PATTERN)]:
                nc.vector.scalar_tensor_tensor(
                    out=xw[:, c, :], in0=xs[:, c, 0:H], scalar=1.0, in1=w,
                    op0=MULT, op1=MULT, accum_out=scol[:, c : c + 1],
                )
            else:
                nc.vector.tensor_tensor(
                    out=xw[:, c, :], in0=xs[:, c, 0:H], in1=w, op=MULT
                )
                nc.scalar.activation(
                    xdump[:], xw[:, c, :], COPY,
                    accum_out=scol[:, c : c + 1],
                )

        def emit_at(c):
            if AT_ON_ACT:
                nc.scalar.activation(
                    at[:, c, :], sel[:, c, :], COPY,
                    bias=0.0, scale=es[:, c : c + 1],
                )
            else:
                nc.vector.tensor_scalar(
                    out=at[:, c, :], in0=sel[:, c, :],
                    scalar1=es[:, c : c + 1], scalar2=None, op0=MULT,
                )

        def emit_mm(c):
            first, last = c == 0, c == T - 1
            nc.tensor.matmul(
                acc[0:M_OUT, 0:512], at[:, c, :], xs[:, c, 0:512],
                start=first, stop=last,
            )
            nc.tensor.matmul(
                acc[0:M_OUT, 512 : H + 1], at[:, c, :], xs[:, c, 512 : H + 1],
                start=first, stop=last,
            )

        for c0 in range(0, T, 2):
            c1 = min(c0 + 2, T)
            for c in range(c0, c1):
                emit_scores(c)
            nc.scalar.activation(es[:, c0:c1], scol[:, c0:c1], EXP)
            for c in range(c0, c1):
                emit_at(c)
                emit_mm(c)

        de = sc.tile([M_OUT, 1], f32)
        nc.vector.tensor_scalar(
            out=de[:], in0=acc[0:M_OUT, H : H + 1], scalar1=DEN_EPS,
            scalar2=None, op0=ADD,
        )
        rec = sc.tile([M_OUT, 1], f32)
        nc.vector.reciprocal(rec[:], de[:])
        do = sc.tile([M_OUT, H], bf16)
        nc.scalar.activation(
            do[:], acc[0:M_OUT, 0:H], COPY, bias=0.0, scale=rec[:, 0:1]
        )
        nc.scalar.dma_start(out=out_d[:], in_=do[:])

    nc.compile()
    return nc


def _prepare(query_len, seq_lens):
    """Assign examples to cores (balanced stream length); compute T, QB."""
    ql = np.asarray(query_len).astype(np.int64)
    sl = np.asarray(seq_lens).astype(np.int64)
    S = sl.sum(axis=1)
    dl = (sl > 0).sum(axis=1)

    def greedy(metric):
        order = np.argsort(-metric, kind="stable")
        loads = np.zeros(NCORES, np.int64)
        counts = np.zeros(NCORES, np.int64)
        assign = [[] for _ in range(NCORES)]
        for e in order:
            cand = [c for c in range(NCORES) if counts[c] < EX_PER_CORE]
            c = min(cand, key=lambda k: loads[k])
            assign[c].append(int(e))
            loads[c] += int(metric[e])
            counts[c] += 1
        # query block size: 32-aligned (DVE partition offsets must be)
        QB = -(-max(int(ql[a].sum()) for a in assign) // 32) * 32
        T = max(int((QB + int(S[a].sum()) + 127) // 128) for a in assign)
        return assign, T, QB

    best = None
    for metric in (S + ql, S):
        assign, T, QB = greedy(metric)
        if best is None or (T, QB) < (best[1], best[2]):
            best = (assign, T, QB)
    assign, T, QB = best
    for c in range(NCORES):
        m_used = EX_PER_CORE + int(dl[assign[c]].sum())
        assert m_used <= M_OUT, f"core {c} needs {m_used} output rows"
    return assign, T, QB, ql, sl, S, dl


def _pack_core(hs, examples, T, QB, ql, sl, S, dl):
    """Build one core's packed stream, selector and output-row maps."""
    rows = T * 128
    xsh = np.zeros((rows, HP), np.float32)
    xsh[:, H] = 1.0
    sel = np.zeros((rows, M_OUT), np.float32)
    sent_rows = {}  # (e, j) -> output row m
    q_rows = {}  # e -> output row m
    mcol = EX_PER_CORE
    qpos = 0
    pos = QB
    for i, e in enumerate(examples):
        nq = int(ql[e])
        xsh[qpos : qpos + nq, 0:H] = hs[e, 1 : 1 + nq, :]
        sel[qpos : qpos + nq, i] = 1.0
        q_rows[e] = i
        qpos += nq
        ns = int(S[e])
        base = int(ql[e]) + 2
        xsh[pos : pos + ns, 0:H] = hs[e, base : base + ns, :]
        off = 0
        for j in range(int(dl[e])):
            ln = int(sl[e, j])
            sel[pos + off : pos + off + ln, mcol] = 1.0
            sent_rows[(e, j)] = mcol
            off += ln
            mcol += 1
        pos += ns
    seldt = FP8 if SEL_FP8 else BF16
    xs = np.ascontiguousarray(
        xsh.reshape(T, 128, HP).transpose(1, 0, 2)
    ).astype(BF16)
    selr = np.ascontiguousarray(
        sel.reshape(T, 128, M_OUT).transpose(1, 0, 2)
    ).astype(seldt)
    return xs, selr, q_rows, sent_rows


def kernel(hidden_states, W_doc, b_doc, W_query, b_query, query_len, seq_lens):
    hs = np.ascontiguousarray(np.asarray(hidden_states, dtype=np.float32))
    wd = np.asarray(W_doc, np.float32).reshape(H)
    wq = np.asarray(W_query, np.float32).reshape(H)

    assign, T, QB, ql, sl, S, dl = _prepare(query_len, seq_lens)

    nc = _compiled.get((T, QB))
    if nc is None:
        nc = _build(T, QB)
        _compiled[(T, QB)] = nc

    wb = np.stack([wd, wq]).astype(BF16)  # [2, H]; device broadcasts

    in_maps = []
    maps = []
    for c in range(NCORES):
        xs, selr, q_rows, sent_rows = _pack_core(
            hs, assign[c], T, QB, ql, sl, S, dl
        )
        in_maps.append({"xs": xs, "sel": selr, "wb": wb})
        maps.append((q_rows, sent_rows))

    from concourse.bass_utils import run_bass_kernel_spmd

    res = run_bass_kernel_spmd(nc, in_maps, list(range(NCORES)))

    doc = np.zeros((B, D, H), np.float32)
    qp = np.empty((B, H), np.float32)
    for c in range(NCORES):
        r = np.asarray(res.results[c]["out"], dtype=np.float32)
        q_rows, sent_rows = maps[c]
        for e, m in q_rows.items():
            qp[e] = r[m]
        for (e, j), m in sent_rows.items():
            doc[e, j] = r[m]
    q_bcast = np.broadcast_to(qp[:, None, :], (B, D, H))
    return doc, q_bcast
